# revision 1
# baseline (speedup 1.0000x reference)
"""Conv1d (B=32, C_in=C_out=64, L=16384, K=3, VALID) on 8 trn2 cores.

Strategy: data-parallel over batch (4 batches/core). Each core views its
shard as 2 "pairs" of batches stacked into 128 partitions. The conv is
3 PSUM-accumulated matmuls (one per tap) against a block-diagonal
weight lhsT [128, 128] = diag(W_k^T, W_k^T), so one matmul computes two
batches at full 128-partition PE utilization. Accumulation is fp32 in
PSUM; I/O streams are fp16 to halve HBM traffic (the memory roofline).
Bias is fused into the PSUM->SBUF copy. Shapes hardcoded from the spec.
"""

import os

import numpy as np

from concourse import bacc, bass, mybir, tile
from concourse.bass_utils import run_bass_kernel_spmd

B, C, L, K = 32, 64, 16384, 3
LOUT = L - K + 1  # 16382
NCORES = 8
BPC = B // NCORES  # 4 batches per core
PAIRS = BPC // 2  # 2 stacked pairs per core
P = 128  # partitions (2 x C)
NJ = 512  # PSUM inner chunk (one fp32 bank)

F32 = mybir.dt.float32

# precision mode: f16 I/O (default, ~3e-4 rel err) or f32r / f32
MODE = os.environ.get("CONV_MODE", "f16")
CH = int(os.environ.get("CONV_CH", "4096" if MODE == "f16" else "2048"))
BUFS = int(os.environ.get("CONV_BUFS", "6"))
WARMUP = int(os.environ.get("CONV_WARMUP", "8"))

_NC_CACHE = []


def _io_dtypes():
    if MODE == "f16":
        return mybir.dt.float16, mybir.dt.float16, np.float16
    if MODE == "f32r":
        return mybir.dt.float32r, F32, np.float32
    return F32, F32, np.float32


def _build_nc():
    FIN, FOUT, _ = _io_dtypes()
    nc = bacc.Bacc("TRN2", target_bir_lowering=False, debug=False,
                   num_devices=NCORES)

    x2 = nc.dram_tensor("x2", [PAIRS, P, L], FIN, kind="ExternalInput")
    wT = nc.dram_tensor("wT", [P, K, P], FIN, kind="ExternalInput")
    b2 = nc.dram_tensor("b2", [P, 1], F32, kind="ExternalInput")
    y2 = nc.dram_tensor("y2", [PAIRS, P, LOUT], FOUT, kind="ExternalOutput")

    with tile.TileContext(nc) as tc:
        with (
            tc.tile_pool(name="const", bufs=1) as const_pool,
            tc.tile_pool(name="inp", bufs=BUFS) as inp_pool,
            tc.tile_pool(name="outp", bufs=BUFS) as outp_pool,
            tc.tile_pool(name="psum", bufs=8, space=bass.MemorySpace.PSUM)
            as psum_pool,
        ):
            w = const_pool.tile([P, K, P], FIN)
            nc.sync.dma_start(out=w[:], in_=wT[:])
            bias = const_pool.tile([P, 1], F32)
            nc.sync.dma_start(out=bias[:], in_=b2[:])

            # HAM warm-up: dummy matmuls on zeroed SBUF while the first
            # input DMA is in flight, so the PE clock gate is at 8/8
            # (2.4 GHz) when real work arrives instead of ramping through
            # the first ~3.4us of it.
            if WARMUP:
                wz = const_pool.tile([P, NJ], FIN)
                nc.vector.memset(wz[:], 0.0)
                for i in range(WARMUP):
                    wp = psum_pool.tile([P, NJ], F32, tag="acc",
                                        name=f"warm{i}")
                    nc.tensor.matmul(wp[:], wz[:, :P], wz[:],
                                     start=True, stop=True)

            # Input DMAs issue from Sync (HWDGE, fast first-byte) so the
            # pipeline fills immediately; output DMAs from GpSimd (SWDGE —
            # its slow start overlaps the fill) so an output waiting on
            # drains never head-of-line blocks input prefetch. Chunk sizes
            # are shaped: small first chunk so compute starts early, small
            # last chunks so the compute-gated tail after the final input
            # is short.
            ramp = [512, 1024, 2048]
            tail_small = [CH // 2, CH // 4]
            rest = LOUT - sum(ramp)
            body = [CH] * (rest // CH)
            last = rest - sum(body)
            rest1 = LOUT - sum(tail_small)
            body1 = [CH] * (rest1 // CH)
            last1 = rest1 - sum(body1)
            chunk_lists = {
                0: ramp + body + [last],
                1: body1 + [last1] + tail_small,
            }
            for p in range(PAIRS):
                l0 = 0
                for n in chunk_lists[p % 2]:
                    nin = n + K - 1  # l0 + nin <= L always (LOUT = L-2)
                    it = inp_pool.tile([P, CH + K - 1], FIN, tag="in")
                    nc.sync.dma_start(out=it[:, :nin],
                                      in_=x2[p, :, l0:l0 + nin])
                    ot = outp_pool.tile([P, CH], FOUT, tag="out")
                    for j0 in range(0, n, NJ):
                        nj = min(NJ, n - j0)
                        pt = psum_pool.tile([P, NJ], F32, tag="acc")
                        for k in range(K):
                            nc.tensor.matmul(
                                pt[:, :nj],
                                w[:, k, :],
                                it[:, j0 + k:j0 + k + nj],
                                start=(k == 0),
                                stop=(k == K - 1),
                            )
                        # psum -> sbuf with fused bias add, split across
                        # ACT and DVE so the bank frees twice as fast
                        h = nj // 2
                        nc.scalar.add(ot[:, j0:j0 + h], pt[:, :h],
                                      add=bias[:, 0:1])
                        nc.vector.tensor_scalar_add(ot[:, j0 + h:j0 + nj],
                                                    pt[:, h:nj],
                                                    bias[:, 0:1])
                    nc.gpsimd.dma_start(out=y2[p, :, l0:l0 + n],
                                        in_=ot[:, :n])
                    l0 += n

    nc.compile()
    return nc


def _get_nc():
    if not _NC_CACHE:
        _NC_CACHE.append(_build_nc())
    return _NC_CACHE[0]


def _prep_weights(weight, bias, np_in):
    wT = np.zeros((P, K, P), np.float32)
    for k in range(K):
        wtk = np.ascontiguousarray(weight[:, :, k].T)  # [C_in, C_out]
        wT[0:C, k, 0:C] = wtk
        wT[C:P, k, C:P] = wtk
    b2 = np.concatenate([bias, bias]).reshape(P, 1).astype(np.float32)
    return wT.astype(np_in), b2


def kernel(x, weight, bias, _want_results=False, **run_kwargs):
    x = np.asarray(x, np.float32)
    weight = np.asarray(weight, np.float32)
    bias = np.asarray(bias, np.float32)
    _, _, np_in = _io_dtypes()
    nc = _get_nc()
    wT, b2 = _prep_weights(weight, bias, np_in)
    in_maps = [
        {
            "x2": np.ascontiguousarray(
                x[BPC * i:BPC * (i + 1)].reshape(PAIRS, P, L)).astype(
                    np_in, copy=False),
            "wT": wT,
            "b2": b2,
        }
        for i in range(NCORES)
    ]
    res = run_bass_kernel_spmd(nc, in_maps, list(range(NCORES)), **run_kwargs)
    out = np.concatenate(
        [
            res.results[i]["y2"].astype(np.float32).reshape(BPC, C, LOUT)
            for i in range(NCORES)
        ],
        axis=0,
    )
    if _want_results:
        return out, res
    return out



# revision 2
# speedup vs baseline: 1.0117x; 1.0117x over previous
"""Conv1d (B=32, C_in=C_out=64, L=16384, K=3, VALID) on 8 trn2 cores.

Strategy: data-parallel over batch (4 batches/core) with polyphase-2
packing. The host splits each batch's signal into even/odd phases
stacked on 128 partitions (xP[0:64]=x[:,0::2], xP[64:128]=x[:,1::2]);
the conv then needs only TWO PSUM-accumulated matmuls per output tile
(vs 3 tap-matmuls unpacked) against block lhsT matrices
  A = [[W0^T, 0], [W1^T, W0^T]],  B = [[W2^T, W1^T], [0, W2^T]]
where pass B reads the same input tile shifted by one packed column.
Each packed output column holds 2 real output columns (even rows 0:64,
odd rows 64:128), so PE work per output column drops 3 -> 2 cycles and
the kernel is cleanly DMA-bound. Accumulation is fp32 in PSUM; I/O is
fp16 to halve HBM traffic (memory roofline). Bias fuses into the
PSUM->SBUF copy, split across ACT and DVE. Host does the (free)
polyphase pack/unpack. Shapes hardcoded from the spec.
"""

import os

import numpy as np

from concourse import bacc, bass, mybir, tile
from concourse.bass_utils import run_bass_kernel_spmd

B, C, L, K = 32, 64, 16384, 3
LOUT = L - K + 1  # 16382
NCORES = 8
BPC = B // NCORES  # 4 batches per core
UNITS = BPC  # one polyphase unit per batch
P = 128  # partitions (2 phases x 64 ch)
T = L // 2  # 8192 packed input cols
U = LOUT // 2  # 8191 packed output cols
NJ = 512  # PSUM inner chunk (one fp32 bank)

F32 = mybir.dt.float32
F16 = mybir.dt.float16

CH = int(os.environ.get("CONV_CH", "4096"))
BUFS = int(os.environ.get("CONV_BUFS", "6"))
WARMUP = int(os.environ.get("CONV_WARMUP", "8"))

_NC_CACHE = []


def _chunk_lists():
    """Per-unit output-chunk schedules: ramp up on the first unit so
    compute + the output stream start early, ramp down on the last so
    the post-input tail is short, big chunks elsewhere for DMA
    efficiency."""
    ramp = [512, 1024, 2048, 4096]
    first = ramp + [U - sum(ramp)]
    mid = [CH] * (U // CH) + ([U % CH] if U % CH else [])
    tail = [4096, 2048, 1024, 512]
    last = tail + [U - sum(tail)]
    lists = {0: first, UNITS - 1: last}
    return [lists.get(u, mid) for u in range(UNITS)]


def _build_nc():
    nc = bacc.Bacc("TRN2", target_bir_lowering=False, debug=False,
                   num_devices=NCORES)

    x2 = nc.dram_tensor("x2", [UNITS, P, T], F16, kind="ExternalInput")
    wT = nc.dram_tensor("wT", [P, 2, P], F16, kind="ExternalInput")
    b2 = nc.dram_tensor("b2", [P, 1], F32, kind="ExternalInput")
    y2 = nc.dram_tensor("y2", [UNITS, P, U], F16, kind="ExternalOutput")

    with tile.TileContext(nc) as tc:
        with (
            tc.tile_pool(name="const", bufs=1) as const_pool,
            tc.tile_pool(name="inp", bufs=BUFS) as inp_pool,
            tc.tile_pool(name="outp", bufs=BUFS) as outp_pool,
            tc.tile_pool(name="psum", bufs=8, space=bass.MemorySpace.PSUM)
            as psum_pool,
        ):
            # weights + bias ride the ACT HWDGE ring so the Sync ring's
            # first input chunk issues with zero head-of-line delay
            w = const_pool.tile([P, 2, P], F16)
            nc.scalar.dma_start(out=w[:], in_=wT[:])
            bias = const_pool.tile([P, 1], F32)
            nc.scalar.dma_start(out=bias[:], in_=b2[:])

            # HAM warm-up: dummy matmuls on zeroed SBUF while the first
            # input DMA is in flight, so the PE clock gate ramps to 8/8
            # before real work arrives.
            if WARMUP:
                wz = const_pool.tile([P, NJ], F16)
                nc.vector.memset(wz[:], 0.0)
                for i in range(WARMUP):
                    wp = psum_pool.tile([P, NJ], F32, tag="acc",
                                        name=f"warm{i}")
                    nc.tensor.matmul(wp[:], wz[:, :P], wz[:],
                                     start=True, stop=True)

            for u, chunks in enumerate(_chunk_lists()):
                l0 = 0
                for n in chunks:
                    it = inp_pool.tile([P, CH + 1], F16, tag="in")
                    nc.sync.dma_start(out=it[:, :n + 1],
                                      in_=x2[u, :, l0:l0 + n + 1])
                    ot = outp_pool.tile([P, CH], F16, tag="out")
                    for j0 in range(0, n, NJ):
                        nj = min(NJ, n - j0)
                        pt = psum_pool.tile([P, NJ], F32, tag="acc")
                        nc.tensor.matmul(pt[:, :nj], w[:, 0, :],
                                         it[:, j0:j0 + nj],
                                         start=True, stop=False)
                        nc.tensor.matmul(pt[:, :nj], w[:, 1, :],
                                         it[:, j0 + 1:j0 + 1 + nj],
                                         start=False, stop=True)
                        # psum -> sbuf with fused bias add, split across
                        # ACT and DVE so the bank frees twice as fast
                        h = nj // 2
                        nc.scalar.add(ot[:, j0:j0 + h], pt[:, :h],
                                      add=bias[:, 0:1])
                        nc.vector.tensor_scalar_add(ot[:, j0 + h:j0 + nj],
                                                    pt[:, h:nj],
                                                    bias[:, 0:1])
                    nc.gpsimd.dma_start(out=y2[u, :, l0:l0 + n],
                                        in_=ot[:, :n])
                    l0 += n

    nc.compile()
    return nc


def _get_nc():
    if not _NC_CACHE:
        _NC_CACHE.append(_build_nc())
    return _NC_CACHE[0]


def _prep_weights(weight, bias):
    w0, w1, w2 = (np.ascontiguousarray(weight[:, :, k].T) for k in range(K))
    wT = np.zeros((P, 2, P), np.float32)
    wT[0:C, 0, 0:C] = w0
    wT[C:P, 0, 0:C] = w1
    wT[C:P, 0, C:P] = w0
    wT[0:C, 1, 0:C] = w2
    wT[0:C, 1, C:P] = w1
    wT[C:P, 1, C:P] = w2
    b2 = np.concatenate([bias, bias]).reshape(P, 1).astype(np.float32)
    return wT.astype(np.float16), b2


def kernel(x, weight, bias, _want_results=False, **run_kwargs):
    x = np.asarray(x, np.float32)
    weight = np.asarray(weight, np.float32)
    bias = np.asarray(bias, np.float32)
    nc = _get_nc()
    wT, b2 = _prep_weights(weight, bias)
    xP = np.empty((B, P, T), np.float16)
    xP[:, 0:C, :] = x[:, :, 0::2]
    xP[:, C:P, :] = x[:, :, 1::2]
    in_maps = [
        {"x2": np.ascontiguousarray(xP[BPC * i:BPC * (i + 1)]),
         "wT": wT, "b2": b2}
        for i in range(NCORES)
    ]
    res = run_bass_kernel_spmd(nc, in_maps, list(range(NCORES)), **run_kwargs)
    out = np.empty((B, C, LOUT), np.float32)
    for i in range(NCORES):
        yP = res.results[i]["y2"].astype(np.float32)  # [UNITS, P, U]
        out[BPC * i:BPC * (i + 1), :, 0::2] = yP[:, 0:C, :]
        out[BPC * i:BPC * (i + 1), :, 1::2] = yP[:, C:P, :]
    if _want_results:
        return out, res
    return out


# revision 5
# speedup vs baseline: 1.0453x; 1.0332x over previous
"""Conv1d (B=32, C_in=C_out=64, L=16384, K=3, VALID) on 8 trn2 cores.

Strategy: data-parallel over batch (4 batches/core) with polyphase-2
packing. The host splits each batch's signal into even/odd phases
stacked on 128 partitions (xP[0:64]=x[:,0::2], xP[64:128]=x[:,1::2]);
the conv then needs only TWO PSUM-accumulated matmuls per output tile
(vs 3 tap-matmuls unpacked) against block lhsT matrices
  A = [[W0^T, 0], [W1^T, W0^T]],  B = [[W2^T, W1^T], [0, W2^T]]
where pass B reads the same input tile shifted by one packed column.
Each packed output column holds 2 real output columns (even rows 0:64,
odd rows 64:128), so PE work per output column drops 3 -> 2 cycles and
the kernel is cleanly DMA-bound. Accumulation is fp32 in PSUM; I/O is
fp16 to halve HBM traffic (memory roofline). Bias fuses into the
PSUM->SBUF copy, split across ACT and DVE. Host does the (free)
polyphase pack/unpack. Shapes hardcoded from the spec.
"""

import os

import numpy as np

from concourse import bacc, bass, mybir, tile
from concourse.bass_utils import run_bass_kernel_spmd

B, C, L, K = 32, 64, 16384, 3
LOUT = L - K + 1  # 16382
NCORES = 8
BPC = B // NCORES  # 4 batches per core
UNITS = BPC  # one polyphase unit per batch
P = 128  # partitions (2 phases x 64 ch)
T = L // 2  # 8192 packed input cols
U = LOUT // 2  # 8191 packed output cols
NJ = 512  # PSUM inner chunk (one fp32 bank)

F32 = mybir.dt.float32
F16 = mybir.dt.float16

CH = int(os.environ.get("CONV_CH", "2048"))
BUFS_IN = int(os.environ.get("CONV_BUFS_IN", "8"))
BUFS_OUT = int(os.environ.get("CONV_BUFS_OUT", "20"))
WARMUP = int(os.environ.get("CONV_WARMUP", "8"))

_NC_CACHE = []


def _chunk_lists():
    """Per-unit output-chunk schedules: small ramp on the first unit so
    compute + the output stream start early, ramp down on the last so
    the post-input tail is short, CH-sized chunks elsewhere. The output
    tile pool is sized to hold every chunk (BUFS_OUT >= total chunks),
    so drains never block on output-DMA completion."""
    ramp = [512, 1024]
    body = [CH] * ((U - sum(ramp)) // CH)
    first = ramp + body + [U - sum(ramp) - sum(body)]
    mid = [CH] * (U // CH) + ([U % CH] if U % CH else [])
    tail = [1024, 512]
    body = [CH] * ((U - sum(tail)) // CH)
    last = body + [U - sum(tail) - sum(body)] + tail
    lists = {0: first, UNITS - 1: last}
    return [lists.get(u, mid) for u in range(UNITS)]


def _build_nc():
    nc = bacc.Bacc("TRN2", target_bir_lowering=False, debug=False,
                   num_devices=NCORES)

    x2 = nc.dram_tensor("x2", [UNITS, P, T], F16, kind="ExternalInput")
    wT = nc.dram_tensor("wT", [P, 2, P], F16, kind="ExternalInput")
    b2 = nc.dram_tensor("b2", [P, 1], F32, kind="ExternalInput")
    y2 = nc.dram_tensor("y2", [UNITS, P, U], F16, kind="ExternalOutput")

    with tile.TileContext(nc) as tc:
        with (
            tc.tile_pool(name="const", bufs=1) as const_pool,
            tc.tile_pool(name="inp", bufs=BUFS_IN) as inp_pool,
            tc.tile_pool(name="outp", bufs=BUFS_OUT) as outp_pool,
            tc.tile_pool(name="psum", bufs=8, space=bass.MemorySpace.PSUM)
            as psum_pool,
        ):
            # weights + bias ride the ACT HWDGE ring so the Sync ring's
            # first input chunk issues with zero head-of-line delay
            w = const_pool.tile([P, 2, P], F16)
            nc.scalar.dma_start(out=w[:], in_=wT[:])
            bias = const_pool.tile([P, 1], F32)
            nc.scalar.dma_start(out=bias[:], in_=b2[:])

            # HAM warm-up: dummy matmuls on zeroed SBUF while the first
            # input DMA is in flight, so the PE clock gate ramps to 8/8
            # before real work arrives.
            if WARMUP:
                wz = const_pool.tile([P, NJ], F16)
                nc.vector.memset(wz[:], 0.0)
                for i in range(WARMUP):
                    wp = psum_pool.tile([P, NJ], F32, tag="acc",
                                        name=f"warm{i}")
                    nc.tensor.matmul(wp[:], wz[:, :P], wz[:],
                                     start=True, stop=True)

            for u, chunks in enumerate(_chunk_lists()):
                l0 = 0
                for n in chunks:
                    it = inp_pool.tile([P, CH + 1], F16, tag="in")
                    nc.sync.dma_start(out=it[:, :n + 1],
                                      in_=x2[u, :, l0:l0 + n + 1])
                    ot = outp_pool.tile([P, CH], F16, tag="out")
                    for j0 in range(0, n, NJ):
                        nj = min(NJ, n - j0)
                        pt = psum_pool.tile([P, NJ], F32, tag="acc")
                        nc.tensor.matmul(pt[:, :nj], w[:, 0, :],
                                         it[:, j0:j0 + nj],
                                         start=True, stop=False)
                        nc.tensor.matmul(pt[:, :nj], w[:, 1, :],
                                         it[:, j0 + 1:j0 + 1 + nj],
                                         start=False, stop=True)
                        # psum -> sbuf with fused bias add, split across
                        # ACT and DVE so the bank frees twice as fast
                        h = nj // 2
                        nc.scalar.add(ot[:, j0:j0 + h], pt[:, :h],
                                      add=bias[:, 0:1])
                        nc.vector.tensor_scalar_add(ot[:, j0 + h:j0 + nj],
                                                    pt[:, h:nj],
                                                    bias[:, 0:1])
                    nc.gpsimd.dma_start(out=y2[u, :, l0:l0 + n],
                                        in_=ot[:, :n])
                    l0 += n

    nc.compile()
    return nc


def _get_nc():
    if not _NC_CACHE:
        _NC_CACHE.append(_build_nc())
    return _NC_CACHE[0]


def _prep_weights(weight, bias):
    w0, w1, w2 = (np.ascontiguousarray(weight[:, :, k].T) for k in range(K))
    wT = np.zeros((P, 2, P), np.float32)
    wT[0:C, 0, 0:C] = w0
    wT[C:P, 0, 0:C] = w1
    wT[C:P, 0, C:P] = w0
    wT[0:C, 1, 0:C] = w2
    wT[0:C, 1, C:P] = w1
    wT[C:P, 1, C:P] = w2
    b2 = np.concatenate([bias, bias]).reshape(P, 1).astype(np.float32)
    return wT.astype(np.float16), b2


def kernel(x, weight, bias, _want_results=False, **run_kwargs):
    x = np.asarray(x, np.float32)
    weight = np.asarray(weight, np.float32)
    bias = np.asarray(bias, np.float32)
    nc = _get_nc()
    wT, b2 = _prep_weights(weight, bias)
    xP = np.empty((B, P, T), np.float16)
    xP[:, 0:C, :] = x[:, :, 0::2]
    xP[:, C:P, :] = x[:, :, 1::2]
    in_maps = [
        {"x2": np.ascontiguousarray(xP[BPC * i:BPC * (i + 1)]),
         "wT": wT, "b2": b2}
        for i in range(NCORES)
    ]
    res = run_bass_kernel_spmd(nc, in_maps, list(range(NCORES)), **run_kwargs)
    out = np.empty((B, C, LOUT), np.float32)
    for i in range(NCORES):
        yP = res.results[i]["y2"].astype(np.float32)  # [UNITS, P, U]
        out[BPC * i:BPC * (i + 1), :, 0::2] = yP[:, 0:C, :]
        out[BPC * i:BPC * (i + 1), :, 1::2] = yP[:, C:P, :]
    if _want_results:
        return out, res
    return out


# revision 7
# speedup vs baseline: 1.1222x; 1.0736x over previous
"""Conv1d (B=32, C_in=C_out=64, L=16384, K=3, VALID) on 8 trn2 cores.

Strategy: data-parallel over batch (4 batches/core) with polyphase-2
packing. The host splits each batch's signal into even/odd phases
stacked on 128 partitions (xP[0:64]=x[:,0::2], xP[64:128]=x[:,1::2]);
the conv then needs only TWO PSUM-accumulated matmuls per output tile
(vs 3 tap-matmuls unpacked) against block lhsT matrices
  A = [[W0^T, 0], [W1^T, W0^T]],  B = [[W2^T, W1^T], [0, W2^T]]
where pass B reads the same input tile shifted by one packed column.
Each packed output column holds 2 real output columns (even rows 0:64,
odd rows 64:128), so PE work per output column drops 3 -> 2 cycles and
the kernel is cleanly DMA-bound. Accumulation is fp32 in PSUM; I/O is
fp16 to halve HBM traffic (memory roofline). Bias fuses into the
PSUM->SBUF copy, split across ACT and DVE. Host does the (free)
polyphase pack/unpack. Shapes hardcoded from the spec.
"""

import os

import numpy as np

from concourse import bacc, bass, mybir, tile
from concourse.bass_utils import run_bass_kernel_spmd

B, C, L, K = 32, 64, 16384, 3
LOUT = L - K + 1  # 16382
NCORES = 8
BPC = B // NCORES  # 4 batches per core
UNITS = BPC  # one polyphase unit per batch
P = 128  # partitions (2 phases x 64 ch)
T = L // 2  # 8192 packed input cols
U = LOUT // 2  # 8191 packed output cols
NJ = 512  # PSUM inner chunk (one fp32 bank)

F32 = mybir.dt.float32
F16 = mybir.dt.float16

CH = int(os.environ.get("CONV_CH", "2048"))
BUFS_IN = int(os.environ.get("CONV_BUFS_IN", "20"))
BUFS_OUT = int(os.environ.get("CONV_BUFS_OUT", "20"))
WARMUP = int(os.environ.get("CONV_WARMUP", "0"))

_NC_CACHE = []


def _chunk_lists():
    """Per-unit output-chunk schedules: small ramp on the first unit so
    compute + the output stream start early, ramp down on the last so
    the post-input tail is short, CH-sized chunks elsewhere. The output
    tile pool is sized to hold every chunk (BUFS_OUT >= total chunks),
    so drains never block on output-DMA completion."""
    ramp = [512, 1024]
    body = [CH] * ((U - sum(ramp)) // CH)
    first = ramp + body + [U - sum(ramp) - sum(body)]
    mid = [CH] * (U // CH) + ([U % CH] if U % CH else [])
    tail = [1024, 512]
    body = [CH] * ((U - sum(tail)) // CH)
    last = body + [U - sum(tail) - sum(body)] + tail
    lists = {0: first, UNITS - 1: last}
    return [lists.get(u, mid) for u in range(UNITS)]


def _build_nc():
    nc = bacc.Bacc("TRN2", target_bir_lowering=False, debug=False,
                   num_devices=NCORES)

    x2 = nc.dram_tensor("x2", [UNITS, P, T], F16, kind="ExternalInput")
    wT = nc.dram_tensor("wT", [P, 2, P], F16, kind="ExternalInput")
    b2 = nc.dram_tensor("b2", [P, 1], F32, kind="ExternalInput")
    y2 = nc.dram_tensor("y2", [UNITS, P, U], F16, kind="ExternalOutput")

    with tile.TileContext(nc) as tc:
        with (
            tc.tile_pool(name="const", bufs=1) as const_pool,
            tc.tile_pool(name="inp", bufs=BUFS_IN) as inp_pool,
            tc.tile_pool(name="outp", bufs=BUFS_OUT) as outp_pool,
            tc.tile_pool(name="psum", bufs=8, space=bass.MemorySpace.PSUM)
            as psum_pool,
        ):
            # weights + bias ride the ACT HWDGE ring so the Sync ring's
            # first input chunk issues with zero head-of-line delay
            w = const_pool.tile([P, 2, P], F16)
            nc.scalar.dma_start(out=w[:], in_=wT[:])
            bias = const_pool.tile([P, 1], F32)
            nc.scalar.dma_start(out=bias[:], in_=b2[:])

            # HAM warm-up: tiny dummy matmuls so the PE clock gate starts
            # ramping before real work (off by default — compute has big
            # slack vs the DMA roofline, so ramp losses are hidden).
            if WARMUP:
                wz = const_pool.tile([P, NJ], F16)
                nc.vector.memset(wz[:], 0.0)
                for i in range(WARMUP):
                    wp = psum_pool.tile([P, NJ], F32, tag="acc",
                                        name=f"warm{i}")
                    nc.tensor.matmul(wp[:, :64], wz[:, :P], wz[:, :64],
                                     start=True, stop=True)

            # All input chunks + all output chunks are SBUF-resident
            # (BUFS >= chunk count), and input AND output DMAs share the
            # single in-order Sync HWDGE ring with every input emitted
            # first: the input stream gets strict bus priority and runs
            # at full single-stream HBM rate, compute follows with no
            # buffer stalls, and the buffered output then streams out at
            # full rate. This beats fair round-robin of in/out streams,
            # which lets the compute tail get exposed (and HAM-downclocked)
            # at the end.
            outs = []
            for u, chunks in enumerate(_chunk_lists()):
                l0 = 0
                for n in chunks:
                    it = inp_pool.tile([P, CH + 1], F16, tag="in")
                    nc.sync.dma_start(out=it[:, :n + 1],
                                      in_=x2[u, :, l0:l0 + n + 1])
                    ot = outp_pool.tile([P, CH], F16, tag="out")
                    for j0 in range(0, n, NJ):
                        nj = min(NJ, n - j0)
                        pt = psum_pool.tile([P, NJ], F32, tag="acc")
                        nc.tensor.matmul(pt[:, :nj], w[:, 0, :],
                                         it[:, j0:j0 + nj],
                                         start=True, stop=False)
                        nc.tensor.matmul(pt[:, :nj], w[:, 1, :],
                                         it[:, j0 + 1:j0 + 1 + nj],
                                         start=False, stop=True)
                        # psum -> sbuf with fused bias add, split across
                        # ACT and DVE so the bank frees twice as fast
                        h = nj // 2
                        nc.scalar.add(ot[:, j0:j0 + h], pt[:, :h],
                                      add=bias[:, 0:1])
                        nc.vector.tensor_scalar_add(ot[:, j0 + h:j0 + nj],
                                                    pt[:, h:nj],
                                                    bias[:, 0:1])
                    outs.append((u, l0, n, ot))
                    l0 += n
            for u, l0, n, ot in outs:
                nc.sync.dma_start(out=y2[u, :, l0:l0 + n], in_=ot[:, :n])

    nc.compile()
    return nc


def _get_nc():
    if not _NC_CACHE:
        _NC_CACHE.append(_build_nc())
    return _NC_CACHE[0]


def _prep_weights(weight, bias):
    w0, w1, w2 = (np.ascontiguousarray(weight[:, :, k].T) for k in range(K))
    wT = np.zeros((P, 2, P), np.float32)
    wT[0:C, 0, 0:C] = w0
    wT[C:P, 0, 0:C] = w1
    wT[C:P, 0, C:P] = w0
    wT[0:C, 1, 0:C] = w2
    wT[0:C, 1, C:P] = w1
    wT[C:P, 1, C:P] = w2
    b2 = np.concatenate([bias, bias]).reshape(P, 1).astype(np.float32)
    return wT.astype(np.float16), b2


def kernel(x, weight, bias, _want_results=False, **run_kwargs):
    x = np.asarray(x, np.float32)
    weight = np.asarray(weight, np.float32)
    bias = np.asarray(bias, np.float32)
    nc = _get_nc()
    wT, b2 = _prep_weights(weight, bias)
    xP = np.empty((B, P, T), np.float16)
    xP[:, 0:C, :] = x[:, :, 0::2]
    xP[:, C:P, :] = x[:, :, 1::2]
    in_maps = [
        {"x2": np.ascontiguousarray(xP[BPC * i:BPC * (i + 1)]),
         "wT": wT, "b2": b2}
        for i in range(NCORES)
    ]
    res = run_bass_kernel_spmd(nc, in_maps, list(range(NCORES)), **run_kwargs)
    out = np.empty((B, C, LOUT), np.float32)
    for i in range(NCORES):
        yP = res.results[i]["y2"].astype(np.float32)  # [UNITS, P, U]
        out[BPC * i:BPC * (i + 1), :, 0::2] = yP[:, 0:C, :]
        out[BPC * i:BPC * (i + 1), :, 1::2] = yP[:, C:P, :]
    if _want_results:
        return out, res
    return out
